# revision 27
# baseline (speedup 1.0000x reference)
"""Trainium2 Bass kernel for the RPO actor (MLP encoder -> masked LSTM -> Gaussian head).

Sharding: data-parallel over the env dim B (1024 envs -> 128 per NeuronCore),
weights replicated, no collectives. Host does all layout work (transposes,
gate reorder, mask broadcast); device runs feature-major ("transposed")
matmuls + a tanh-only LSTM cell:
    sigmoid(x) = (1 + tanh(x/2)) / 2
so the four gate activations collapse into ONE tanh(0.5*gates) op per step
(g-gate weights are pre-doubled on the host), and the sigmoid affines are
folded into fused scalar_tensor_tensor ops / host-side constant scalings.
"""

import math
import numpy as np
import ml_dtypes
from contextlib import ExitStack

import concourse.bass as bass
import concourse.tile as tile
import concourse.mybir as mybir
from concourse import bacc
from concourse.bass_utils import run_bass_kernel_spmd

OBS, ACT_D = 128, 8
T, B = 256, 1024
H1, H2, HL = 512, 256, 128
NCORES = 8
BS = B // NCORES          # 128 envs per core
ROWS = T * BS             # 32768 rows per core
TC = 64                   # timesteps per chunk
NCHUNK = T // TC
G = 2                     # steps per x-matmul group (must divide TC)
PRIO = 20000               # high-priority offset for the LSTM chain (0=off)

BF16 = mybir.dt.bfloat16
F32 = mybir.dt.float32
NPBF16 = ml_dtypes.bfloat16
Tanh = mybir.ActivationFunctionType.Tanh
ADD = mybir.AluOpType.add
MULT = mybir.AluOpType.mult
SUB = mybir.AluOpType.subtract

_BUILD_CACHE = {}


def _build(with_bias: bool, nchunk: int = NCHUNK):
    nc = bacc.Bacc("TRN2", target_bir_lowering=False, debug=False,
                   num_devices=NCORES)
    P = lambda n, s, d, o=False: nc.declare_dram_parameter(n, s, d, isOutput=o)
    stateT = P("stateT", [OBS, ROWS], BF16)
    mhalf = P("mhalf", [128, (T + 1) * BS], BF16)   # 0.5*(1-done), bcast over partitions
    az2s = P("az2s", [BS, T * ACT_D], F32)          # (action - z - bmean)/std, env-major
    w1t = P("w1t", [OBS, H1], BF16)
    w2t = P("w2t", [H1, H2], BF16)
    wiht = P("wiht", [H2, 4 * HL], BF16)            # gate cols (i,f,o,2g)
    whht = P("whht", [HL, 4 * HL], BF16)
    biasr = P("biasr", [1, 4 * HL], BF16)           # (bih+bhh) reordered, g doubled
    wmsh = P("wmsh", [HL, ACT_D], BF16)             # (Wmean/std).T / 2
    h0T = P("h0T", [HL, BS], F32)
    c0T = P("c0T", [HL, BS], F32)
    lp_o = P("lp", [BS, T], F32, True)              # sum-of-squares, env-major
    h_o = P("h_o", [HL, BS], F32, True)
    c_o = P("c_o", [HL, BS], F32, True)

    with tile.TileContext(nc) as tc:
        with ExitStack() as ctx:
            const = ctx.enter_context(tc.tile_pool(name="const", bufs=1))
            stream = ctx.enter_context(tc.tile_pool(name="stream", bufs=2))
            xpool = ctx.enter_context(tc.tile_pool(name="xpool", bufs=2))
            h1pool = ctx.enter_context(tc.tile_pool(name="h1pool", bufs=3))
            work = ctx.enter_context(tc.tile_pool(name="work", bufs=4))
            statep = ctx.enter_context(tc.tile_pool(name="statep", bufs=3))
            mlp_ps = ctx.enter_context(
                tc.tile_pool(name="mlp_ps", bufs=1, space="PSUM"))
            gate_ps = ctx.enter_context(
                tc.tile_pool(name="gate_ps", bufs=2, space="PSUM"))
            mean_psp = ctx.enter_context(
                tc.tile_pool(name="mean_ps", bufs=2, space="PSUM"))

            # ---- load constants ----
            w1_sb = const.tile([OBS, H1], BF16, tag="w1")
            nc.gpsimd.dma_start(w1_sb[:], w1t[:])
            w2_sb = [const.tile([128, H2], BF16, tag=f"w2_{k}", name=f"w2_{k}")
                     for k in range(4)]
            for k in range(4):
                nc.gpsimd.dma_start(w2_sb[k][:], w2t[k * 128:(k + 1) * 128, :])
            wih_sb = [const.tile([128, 4 * HL], BF16, tag=f"wih_{k}", name=f"wih_{k}")
                      for k in range(2)]
            for k in range(2):
                nc.gpsimd.dma_start(wih_sb[k][:], wiht[k * 128:(k + 1) * 128, :])
            whh_sb = const.tile([HL, 4 * HL], BF16, tag="whh")
            nc.gpsimd.dma_start(whh_sb[:], whht[:])
            wms_sb = const.tile([HL, ACT_D], BF16, tag="wms")
            nc.gpsimd.dma_start(wms_sb[:], wmsh[:])
            az_sb = const.tile([BS, T, ACT_D], F32, tag="az")
            nc.gpsimd.dma_start(az_sb[:], az2s[:])
            h0_sb = const.tile([HL, BS], F32, tag="h0")
            nc.gpsimd.dma_start(h0_sb[:], h0T[:])
            c0_sb = const.tile([HL, BS], F32, tag="c0")
            nc.gpsimd.dma_start(c0_sb[:], c0T[:])
            lp_sb = const.tile([BS, T], F32, tag="lp")
            if with_bias:
                bias_sb = const.tile([1, 4 * HL], BF16, tag="bias")
                nc.gpsimd.dma_start(bias_sb[:], biasr[:])
                ones_sb = const.tile([1, G * BS], BF16, tag="ones")
                nc.vector.memset(ones_sb[:], 1.0)

            c_prev = c0_sb
            hm_prev = None  # set after mask of chunk 0 is loaded
            mean_ps = None
            Tt_last = None
            thc_last = None

            for c in range(nchunk):
                rl0 = c * TC * BS  # first row (in ROWS) of this chunk
                # ---- stream in state + mask ----
                stT = stream.tile([OBS, TC * BS], BF16, tag="stT")
                nc.gpsimd.dma_start(stT[:], stateT[:, rl0:rl0 + TC * BS])
                mh = stream.tile([128, (TC + 1) * BS], BF16, tag="mh")
                nc.gpsimd.dma_start(mh[:], mhalf[:, rl0:rl0 + (TC + 1) * BS])

                if c == 0:
                    # hm_0 = (2*h0) * mhalf[0]  (= h0 * mask)
                    hm0 = work.tile([HL, BS], BF16, tag="hm")
                    nc.vector.scalar_tensor_tensor(
                        hm0[:], h0_sb[:], 2.0, mh[:, 0:BS], op0=MULT, op1=MULT)
                    hm_prev = hm0

                # ---- MLP, streamed per 1024-row tile (layer1 then layer2) ----
                xT = [xpool.tile([128, TC * BS], BF16, tag=f"xt{jj}", name=f"xt{jj}")
                      for jj in range(2)]
                NQ = TC * BS // 1024
                for q in range(NQ):
                    h1q = h1pool.tile([128, 4, 1024], BF16, tag="h1q")
                    for j in range(4):
                        ps = mlp_ps.tile([128, 1024], F32, tag="mlp")
                        for s in range(2):
                            nc.tensor.matmul(
                                ps[:, s * 512:(s + 1) * 512],
                                w1_sb[:, j * 128:(j + 1) * 128],
                                stT[:, q * 1024 + s * 512:q * 1024 + (s + 1) * 512],
                                start=True, stop=True)
                        for s in range(2):
                            nc.scalar.activation(
                                h1q[:, j, s * 512:(s + 1) * 512],
                                ps[:, s * 512:(s + 1) * 512], Tanh)
                    for jj in range(2):
                        ps = mlp_ps.tile([128, 1024], F32, tag="mlp")
                        for s in range(2):
                            for k in range(4):
                                nc.tensor.matmul(
                                    ps[:, s * 512:(s + 1) * 512],
                                    w2_sb[k][:, jj * 128:(jj + 1) * 128],
                                    h1q[:, k, s * 512:(s + 1) * 512],
                                    start=(k == 0), stop=(k == 3))
                        for s in range(2):
                            nc.scalar.activation(
                                xT[jj][:, q * 1024 + s * 512:q * 1024 + (s + 1) * 512],
                                ps[:, s * 512:(s + 1) * 512], Tanh)

                # ---- LSTM scan over this chunk ----
                for tl0 in range(0, TC, G):
                    # gates psum: [feat, gate j, (dt, b)] ; x-part batched over G steps
                    # One accumulation group per PSUM bank: start=True only on
                    # the first matmul touching the bank (it clears has_written
                    # for the WHOLE bank), stop=True on the bank's last matmul
                    # (the final recurrent matmul below). With G=2, gates (i,f)
                    # share bank 0 and (o,g) share bank 1.
                    ps = gate_ps.tile([128, 4, G * BS], F32, tag="gates")
                    for j in range(4):
                        for k in range(2):
                            nc.tensor.matmul(
                                ps[:, j, :],
                                wih_sb[k][:, j * 128:(j + 1) * 128],
                                xT[k][:, tl0 * BS:(tl0 + G) * BS],
                                start=(k == 0 and j % 2 == 0), stop=False,
                                skip_group_check=True)
                        if with_bias:
                            nc.tensor.matmul(
                                ps[:, j, :],
                                bias_sb[:, j * 128:(j + 1) * 128],
                                ones_sb[:],
                                start=False, stop=False,
                                skip_group_check=True)
                    for dt in range(G):
                        tl = tl0 + dt
                        t = c * TC + tl
                        hp_ctx = tc.high_priority(offset=PRIO) if PRIO else None
                        if hp_ctx: hp_ctx.__enter__()
                        # Tt layout: [i, f, g, o, cm] along dim1. cm (the
                        # masked cell, bf16) rides in slot 4 so one fused stt
                        # computes both (1+Ti)*Tg and (1+Tf)*cm via the
                        # strided in1 AP (g at slot 2, cm at slot 4).
                        Tt = work.tile([128, 5, BS], BF16, tag="Tt")
                        nc.vector.tensor_tensor(
                            Tt[:, 4, :], c_prev[:], mh[:, tl * BS:(tl + 1) * BS],
                            op=MULT)
                        # recurrent matmuls
                        for j in range(4):
                            nc.tensor.matmul(
                                ps[:, j, dt * BS:(dt + 1) * BS],
                                whh_sb[:, j * 128:(j + 1) * 128],
                                hm_prev[:],
                                start=False,
                                stop=(dt == G - 1 and j % 2 == 1),
                                skip_group_check=True)
                        # one tanh for all gates: T = tanh(0.5 * gates)
                        nc.scalar.activation(
                            Tt[:, 0:4, :], ps[:, 0:4, dt * BS:(dt + 1) * BS],
                            Tanh, scale=0.5)
                        Ti, Tf, Tg, To = (Tt[:, 0, :], Tt[:, 1, :],
                                          Tt[:, 2, :], Tt[:, 3, :])
                        # u0 = (1+Ti)*Tg ; u1 = (1+Tf)*cm  (one fused stt)
                        u = work.tile([128, 2, BS], BF16, tag="u")
                        nc.vector.scalar_tensor_tensor(
                            u[:], Tt[:, 0:2, :], 1.0, Tt[:, 2:5:2, :],
                            op0=ADD, op1=MULT)
                        # c_new = u1 + 0.5*u0
                        c_new = statep.tile([HL, BS], F32, tag="c")
                        nc.vector.scalar_tensor_tensor(
                            c_new[:], u[:, 0, :], 0.5, u[:, 1, :],
                            op0=MULT, op1=ADD)
                        thc = work.tile([HL, BS], BF16, tag="thc")
                        nc.scalar.activation(thc[:], c_new[:], Tanh)
                        # om = (1+To) * mask/2 ; hm = om * thc ( = sig(o)*mask*tanh(c))
                        om = work.tile([HL, BS], BF16, tag="om")
                        nc.vector.scalar_tensor_tensor(
                            om[:], To, 1.0, mh[:, (tl + 1) * BS:(tl + 2) * BS],
                            op0=ADD, op1=MULT)
                        hm = work.tile([HL, BS], BF16, tag="hm")
                        nc.vector.tensor_tensor(hm[:], om[:], thc[:], op=MULT)
                        # hp = (1+To)*thc = 2*h  (head lhsT; Wmean pre-halved)
                        hp = work.tile([HL, BS], BF16, tag="hp")
                        nc.vector.scalar_tensor_tensor(
                            hp[:], To, 1.0, thc[:], op0=ADD, op1=MULT)
                        # head matmul: mean/std accumulated per 64-step epoch
                        tmod = t % 64
                        if tmod == 0:
                            mean_ps = mean_psp.tile([BS, 64, ACT_D], F32,
                                                    tag="mean")
                        nc.tensor.matmul(
                            mean_ps[:, tmod, :], hp[:], wms_sb[:],
                            start=True, stop=True)
                        if tmod == 63:
                            e = t // 64
                            diff = work.tile([BS, 64, ACT_D], F32, tag="diff")
                            nc.vector.tensor_tensor(
                                diff[:], az_sb[:, e * 64:(e + 1) * 64, :],
                                mean_ps[:], op=SUB)
                            sq = work.tile([BS, 64, ACT_D], BF16, tag="sq")
                            nc.vector.tensor_tensor(
                                sq[:], diff[:], diff[:], op=MULT)
                            nc.vector.tensor_reduce(
                                lp_sb[:, e * 64:(e + 1) * 64], sq[:],
                                axis=mybir.AxisListType.X, op=ADD)
                        if hp_ctx: hp_ctx.__exit__(None, None, None)
                        c_prev = c_new
                        hm_prev = hm
                        if t == nchunk * TC - 1:
                            Tt_last, thc_last = Tt, thc

            # ---- final outputs ----
            # h_T = 0.5*(1+To)*tanh(c_T), recomputed in f32 off the last step
            thc32 = work.tile([HL, BS], F32, tag="thc32")
            nc.scalar.activation(thc32[:], c_prev[:], Tanh)
            hf1 = work.tile([HL, BS], F32, tag="hf1")
            nc.vector.scalar_tensor_tensor(
                hf1[:], Tt_last[:, 3, :], 1.0, thc32[:], op0=ADD, op1=MULT)
            hf = work.tile([HL, BS], F32, tag="hf")
            nc.vector.tensor_scalar_mul(hf[:], hf1[:], 0.5)
            nc.gpsimd.dma_start(h_o[:], hf[:])
            nc.gpsimd.dma_start(c_o[:], c_prev[:])
            ncol = max(64, (nchunk * TC // 64) * 64)
            nc.gpsimd.dma_start(lp_o[:, 0:ncol], lp_sb[:, 0:ncol])

    nc.finalize()
    return nc


def get_nc(with_bias: bool, nchunk: int = NCHUNK):
    key = (bool(with_bias), nchunk)
    if key not in _BUILD_CACHE:
        _BUILD_CACHE[key] = _build(with_bias, nchunk)
    return _BUILD_CACHE[key]


def _prep_core_inputs(inputs):
    """Host-side layout prep. Returns (in_maps, host_ctx)."""
    state = np.asarray(inputs["state"], np.float32)
    done = np.asarray(inputs["done"], np.float32)
    h0 = np.asarray(inputs["h0"], np.float32)
    c0 = np.asarray(inputs["c0"], np.float32)
    action = np.asarray(inputs["action"], np.float32)
    z = np.asarray(inputs["z"], np.float32)
    W1 = np.asarray(inputs["W1"], np.float32)
    W2 = np.asarray(inputs["W2"], np.float32)
    Wih = np.asarray(inputs["Wih"], np.float32)
    Whh = np.asarray(inputs["Whh"], np.float32)
    bih = np.asarray(inputs["bih"], np.float32)
    bhh = np.asarray(inputs["bhh"], np.float32)
    Wmean = np.asarray(inputs["Wmean"], np.float32)
    bmean = np.asarray(inputs["bmean"], np.float32)
    logstd = np.asarray(inputs["logstd"], np.float32)

    std = np.exp(logstd[0])                      # [8]
    perm = [0, 1, 2, 3]                          # torch order kept: (i,f,g,o)
    gscale = np.array([1.0, 1.0, 2.0, 1.0], np.float32)[:, None]

    wih_r = Wih.reshape(4, HL, H2)[perm] * gscale[:, :, None]
    whh_r = Whh.reshape(4, HL, HL)[perm] * gscale[:, :, None]
    bias_r = ((bih + bhh).reshape(4, HL)[perm] * gscale).reshape(1, 4 * HL)
    with_bias = bool(np.any(bias_r != 0.0))

    wiht = np.ascontiguousarray(wih_r.reshape(4 * HL, H2).T).astype(NPBF16)
    whht = np.ascontiguousarray(whh_r.reshape(4 * HL, HL).T).astype(NPBF16)
    biasr = bias_r.astype(NPBF16)
    w1t = np.ascontiguousarray(W1.T).astype(NPBF16)         # [OBS, H1]
    w2t = np.ascontiguousarray(W2.T).astype(NPBF16)         # [H1, H2]
    wmsh = np.ascontiguousarray((Wmean / (2.0 * std[:, None])).T).astype(NPBF16)

    st = state.reshape(T, B, OBS)
    dn = done.reshape(T, B)
    ac = action.reshape(T, B, ACT_D)
    zz = z.reshape(T, B, ACT_D)

    in_maps = []
    for core in range(NCORES):
        sl = slice(core * BS, (core + 1) * BS)
        stateT = np.ascontiguousarray(
            st[:, sl, :].transpose(2, 0, 1).reshape(OBS, ROWS)).astype(NPBF16)
        mrow = 0.5 * (1.0 - dn[:, sl]).reshape(ROWS)
        mrow = np.concatenate([mrow, np.full(BS, 0.5, np.float32)])
        mhalf = np.ascontiguousarray(
            np.broadcast_to(mrow[None, :], (128, ROWS + BS))).astype(NPBF16)
        az = (ac[:, sl, :] - zz[:, sl, :] - bmean[None, None, :]) / std
        az2s = np.ascontiguousarray(
            az.transpose(1, 0, 2).reshape(BS, T * ACT_D)).astype(np.float32)
        in_maps.append({
            "stateT": stateT, "mhalf": mhalf, "az2s": az2s,
            "w1t": w1t, "w2t": w2t, "wiht": wiht, "whht": whht,
            "biasr": biasr, "wmsh": wmsh,
            "h0T": np.ascontiguousarray(h0[0, sl, :].T).astype(np.float32),
            "c0T": np.ascontiguousarray(c0[0, sl, :].T).astype(np.float32),
        })
    host_ctx = dict(action=inputs["action"], logstd=logstd, with_bias=with_bias)
    return in_maps, host_ctx


def _assemble(results, host_ctx):
    logstd = host_ctx["logstd"]
    LOG2PI = math.log(2.0 * math.pi)
    const = float(np.sum(logstd) + 0.5 * ACT_D * LOG2PI)
    lp = np.empty((T, B), np.float32)
    hT = np.empty((1, B, HL), np.float32)
    cT = np.empty((1, B, HL), np.float32)
    for core in range(NCORES):
        sl = slice(core * BS, (core + 1) * BS)
        r = results[core]
        lp[:, sl] = r["lp"].T
        hT[0, sl, :] = r["h_o"].T
        cT[0, sl, :] = r["c_o"].T
    logprob = (-0.5 * lp.reshape(T * B)) - const
    ent_row = float(np.sum(0.5 + 0.5 * LOG2PI + logstd))
    entropy = np.full(T * B, ent_row, np.float32)
    action = np.asarray(host_ctx["action"], np.float32)
    return action, logprob.astype(np.float32), entropy, hT, cT


def _run_device(in_maps, with_bias):
    nc = get_nc(with_bias)
    res = run_bass_kernel_spmd(nc, in_maps, core_ids=list(range(NCORES)))
    return res.results


def _worker_main(in_pkl, out_pkl):
    import pickle
    with open(in_pkl, "rb") as f:
        d = pickle.load(f)
    results = _run_device(d["in_maps"], d["with_bias"])
    with open(out_pkl, "wb") as f:
        pickle.dump(results, f)


def kernel(**inputs):
    """Full-input entry point. Tries an in-process device run first; on an
    (intermittent) NRT fault the accelerator is unrecoverable for the owning
    process, so it falls back to fresh-subprocess retries."""
    import os
    import pickle
    import subprocess
    import sys
    import tempfile
    import time

    in_maps, host_ctx = _prep_core_inputs(inputs)
    with tempfile.TemporaryDirectory() as td:
        in_pkl = os.path.join(td, "in.pkl")
        out_pkl = os.path.join(td, "out.pkl")
        with open(in_pkl, "wb") as f:
            pickle.dump({"in_maps": in_maps,
                         "with_bias": host_ctx["with_bias"]}, f)
        # fast path: run in-process (build is cached for repeat calls)
        try:
            results = _run_device(in_maps, host_ctx["with_bias"])
            return _assemble(results, host_ctx)
        except Exception as e:
            last_err = e
        # device fault: the owning process can't recover the accelerator;
        # retry in fresh subprocesses after letting the pool reset
        for attempt in range(2):
            time.sleep(75)
            try:
                proc = subprocess.run(
                    [sys.executable, os.path.abspath(__file__),
                     "--worker", in_pkl, out_pkl],
                    timeout=1800, capture_output=True, text=True)
                if proc.returncode == 0 and os.path.exists(out_pkl):
                    with open(out_pkl, "rb") as f:
                        results = pickle.load(f)
                    return _assemble(results, host_ctx)
                last_err = RuntimeError(
                    f"worker rc={proc.returncode}: {proc.stderr[-2000:]}")
            except Exception as e:
                last_err = e
        raise last_err


if __name__ == "__main__":
    import sys
    if len(sys.argv) == 4 and sys.argv[1] == "--worker":
        _worker_main(sys.argv[2], sys.argv[3])


# revision 33
# speedup vs baseline: 1.0126x; 1.0126x over previous
"""Trainium2 Bass kernel for the RPO actor (MLP encoder -> masked LSTM -> Gaussian head).

Sharding: data-parallel over the env dim B (1024 envs -> 128 per NeuronCore),
weights replicated, no collectives. Host does all layout work (transposes,
gate reorder, mask broadcast); device runs feature-major ("transposed")
matmuls + a tanh-only LSTM cell:
    sigmoid(x) = (1 + tanh(x/2)) / 2
so the four gate activations collapse into ONE tanh(0.5*gates) op per step
(g-gate weights are pre-doubled on the host), and the sigmoid affines are
folded into fused scalar_tensor_tensor ops / host-side constant scalings.
"""

import math
import numpy as np
import ml_dtypes
from contextlib import ExitStack

import concourse.bass as bass
import concourse.tile as tile
import concourse.mybir as mybir
from concourse import bacc
from concourse.bass_utils import run_bass_kernel_spmd

OBS, ACT_D = 128, 8
T, B = 256, 1024
H1, H2, HL = 512, 256, 128
NCORES = 8
BS = B // NCORES          # 128 envs per core
ROWS = T * BS             # 32768 rows per core
TC = 64                   # timesteps per chunk
NCHUNK = T // TC
G = 2                     # steps per x-matmul group (must divide TC)
PRIO = 20000               # high-priority offset for the LSTM chain (0=off)

BF16 = mybir.dt.bfloat16
F32 = mybir.dt.float32
NPBF16 = ml_dtypes.bfloat16
Tanh = mybir.ActivationFunctionType.Tanh
ADD = mybir.AluOpType.add
MULT = mybir.AluOpType.mult
SUB = mybir.AluOpType.subtract

_BUILD_CACHE = {}


def _build(with_bias: bool, nchunk: int = NCHUNK):
    nc = bacc.Bacc("TRN2", target_bir_lowering=False, debug=False,
                   num_devices=NCORES)
    P = lambda n, s, d, o=False: nc.declare_dram_parameter(n, s, d, isOutput=o)
    stateT = P("stateT", [OBS, ROWS], BF16)
    mhalf = P("mhalf", [128, (T + 1) * BS], BF16)   # 0.5*(1-done), bcast over partitions
    az2s = P("az2s", [BS, T * ACT_D], F32)          # (action - z - bmean)/std, env-major
    w1t = P("w1t", [OBS, H1], BF16)
    w2t = P("w2t", [H1, H2], BF16)
    wiht = P("wiht", [H2, 4 * HL], BF16)            # gate cols (i,f,o,2g)
    whht = P("whht", [HL, 4 * HL], BF16)
    biasr = P("biasr", [1, 4 * HL], BF16)           # (bih+bhh) reordered, g doubled
    wmsh = P("wmsh", [HL, ACT_D], BF16)             # (Wmean/std).T / 2
    h0T = P("h0T", [HL, BS], F32)
    c0T = P("c0T", [HL, BS], F32)
    lp_o = P("lp", [BS, T], F32, True)              # sum-of-squares, env-major
    h_o = P("h_o", [HL, BS], F32, True)
    c_o = P("c_o", [HL, BS], F32, True)

    with tile.TileContext(nc) as tc:
        with ExitStack() as ctx:
            const = ctx.enter_context(tc.tile_pool(name="const", bufs=1))
            stream = ctx.enter_context(tc.tile_pool(name="stream", bufs=2))
            xpool = ctx.enter_context(tc.tile_pool(name="xpool", bufs=2))
            h1pool = ctx.enter_context(tc.tile_pool(name="h1pool", bufs=3))
            work = ctx.enter_context(tc.tile_pool(name="work", bufs=4))
            statep = ctx.enter_context(tc.tile_pool(name="statep", bufs=3))
            mlp_ps = ctx.enter_context(
                tc.tile_pool(name="mlp_ps", bufs=1, space="PSUM"))
            gate_ps = ctx.enter_context(
                tc.tile_pool(name="gate_ps", bufs=2, space="PSUM"))
            mean_psp = ctx.enter_context(
                tc.tile_pool(name="mean_ps", bufs=2, space="PSUM"))

            # ---- load constants ----
            w1_sb = const.tile([OBS, H1], BF16, tag="w1")
            nc.sync.dma_start(w1_sb[:], w1t[:])
            w2_sb = [const.tile([128, H2], BF16, tag=f"w2_{k}", name=f"w2_{k}")
                     for k in range(4)]
            for k in range(4):
                nc.sync.dma_start(w2_sb[k][:], w2t[k * 128:(k + 1) * 128, :])
            wih_sb = [const.tile([128, 4 * HL], BF16, tag=f"wih_{k}", name=f"wih_{k}")
                      for k in range(2)]
            for k in range(2):
                nc.sync.dma_start(wih_sb[k][:], wiht[k * 128:(k + 1) * 128, :])
            whh_sb = const.tile([HL, 4 * HL], BF16, tag="whh")
            nc.sync.dma_start(whh_sb[:], whht[:])
            wms_sb = const.tile([HL, ACT_D], BF16, tag="wms")
            az_sb = const.tile([BS, T, ACT_D], F32, tag="az")
            h0_sb = const.tile([HL, BS], F32, tag="h0")
            c0_sb = const.tile([HL, BS], F32, tag="c0")
            lp_sb = const.tile([BS, T], F32, tag="lp")
            if with_bias:
                bias_sb = const.tile([1, 4 * HL], BF16, tag="bias")
                nc.sync.dma_start(bias_sb[:], biasr[:])
                ones_sb = const.tile([1, G * BS], BF16, tag="ones")
                nc.vector.memset(ones_sb[:], 1.0)

            c_prev = c0_sb
            hm_prev = None  # set after mask of chunk 0 is loaded
            mean_ps = None
            Tt_last = None
            thc_last = None

            for c in range(nchunk):
                rl0 = c * TC * BS  # first row (in ROWS) of this chunk
                # ---- stream in state + mask ----
                stT = stream.tile([OBS, TC * BS], BF16, tag="stT")
                nc.sync.dma_start(stT[:], stateT[:, rl0:rl0 + TC * BS])
                mh = stream.tile([128, (TC + 1) * BS], BF16, tag="mh")
                nc.sync.dma_start(mh[:], mhalf[:, rl0:rl0 + (TC + 1) * BS])

                if c == 0:
                    # deferred const loads (not needed for the first MLP tiles)
                    nc.sync.dma_start(h0_sb[:], h0T[:])
                    nc.sync.dma_start(c0_sb[:], c0T[:])
                    nc.sync.dma_start(wms_sb[:], wmsh[:])
                    nc.sync.dma_start(az_sb[:], az2s[:])
                    # hm_0 = (2*h0) * mhalf[0]  (= h0 * mask)
                    hm0 = work.tile([HL, BS], BF16, tag="hm")
                    nc.vector.scalar_tensor_tensor(
                        hm0[:], h0_sb[:], 2.0, mh[:, 0:BS], op0=MULT, op1=MULT)
                    hm_prev = hm0

                # ---- MLP, streamed per 1024-row tile (layer1 then layer2) ----
                xT = [xpool.tile([128, TC * BS], BF16, tag=f"xt{jj}", name=f"xt{jj}")
                      for jj in range(2)]
                NQ = TC * BS // 1024
                for q in range(NQ):
                    h1q = h1pool.tile([128, 4, 1024], BF16, tag="h1q")
                    for j in range(4):
                        ps = mlp_ps.tile([128, 1024], F32, tag="mlp")
                        for s in range(2):
                            nc.tensor.matmul(
                                ps[:, s * 512:(s + 1) * 512],
                                w1_sb[:, j * 128:(j + 1) * 128],
                                stT[:, q * 1024 + s * 512:q * 1024 + (s + 1) * 512],
                                start=True, stop=True)
                        for s in range(2):
                            nc.scalar.activation(
                                h1q[:, j, s * 512:(s + 1) * 512],
                                ps[:, s * 512:(s + 1) * 512], Tanh)
                    for jj in range(2):
                        ps = mlp_ps.tile([128, 1024], F32, tag="mlp")
                        for s in range(2):
                            for k in range(4):
                                nc.tensor.matmul(
                                    ps[:, s * 512:(s + 1) * 512],
                                    w2_sb[k][:, jj * 128:(jj + 1) * 128],
                                    h1q[:, k, s * 512:(s + 1) * 512],
                                    start=(k == 0), stop=(k == 3))
                        for s in range(2):
                            nc.scalar.activation(
                                xT[jj][:, q * 1024 + s * 512:q * 1024 + (s + 1) * 512],
                                ps[:, s * 512:(s + 1) * 512], Tanh)

                # ---- LSTM scan over this chunk ----
                for tl0 in range(0, TC, G):
                    # gates psum: [feat, gate j, (dt, b)] ; x-part batched over G steps
                    # One accumulation group per PSUM bank: start=True only on
                    # the first matmul touching the bank (it clears has_written
                    # for the WHOLE bank), stop=True on the bank's last matmul
                    # (the final recurrent matmul below). With G=2, gates (i,f)
                    # share bank 0 and (o,g) share bank 1.
                    ps = gate_ps.tile([128, 4, G * BS], F32, tag="gates")
                    for j in range(4):
                        for k in range(2):
                            nc.tensor.matmul(
                                ps[:, j, :],
                                wih_sb[k][:, j * 128:(j + 1) * 128],
                                xT[k][:, tl0 * BS:(tl0 + G) * BS],
                                start=(k == 0 and j % 2 == 0), stop=False,
                                skip_group_check=True)
                        if with_bias:
                            nc.tensor.matmul(
                                ps[:, j, :],
                                bias_sb[:, j * 128:(j + 1) * 128],
                                ones_sb[:],
                                start=False, stop=False,
                                skip_group_check=True)
                    for dt in range(G):
                        tl = tl0 + dt
                        t = c * TC + tl
                        hp_ctx = tc.high_priority(offset=PRIO) if PRIO else None
                        if hp_ctx: hp_ctx.__enter__()
                        # Tt layout: [i, f, g, o, cm] along dim1. cm (the
                        # masked cell, bf16) rides in slot 4 so one fused stt
                        # computes both (1+Ti)*Tg and (1+Tf)*cm via the
                        # strided in1 AP (g at slot 2, cm at slot 4).
                        Tt = work.tile([128, 5, BS], BF16, tag="Tt")
                        nc.vector.tensor_tensor(
                            Tt[:, 4, :], c_prev[:], mh[:, tl * BS:(tl + 1) * BS],
                            op=MULT)
                        # recurrent matmuls
                        for j in range(4):
                            nc.tensor.matmul(
                                ps[:, j, dt * BS:(dt + 1) * BS],
                                whh_sb[:, j * 128:(j + 1) * 128],
                                hm_prev[:],
                                start=False,
                                stop=(dt == G - 1 and j % 2 == 1),
                                skip_group_check=True)
                        # one tanh for all gates: T = tanh(0.5 * gates)
                        nc.scalar.activation(
                            Tt[:, 0:4, :], ps[:, 0:4, dt * BS:(dt + 1) * BS],
                            Tanh, scale=0.5)
                        Ti, Tf, Tg, To = (Tt[:, 0, :], Tt[:, 1, :],
                                          Tt[:, 2, :], Tt[:, 3, :])
                        # u0 = (1+Ti)*Tg ; u1 = (1+Tf)*cm  (one fused stt)
                        u = work.tile([128, 2, BS], BF16, tag="u")
                        nc.vector.scalar_tensor_tensor(
                            u[:], Tt[:, 0:2, :], 1.0, Tt[:, 2:5:2, :],
                            op0=ADD, op1=MULT)
                        # c_new = u1 + 0.5*u0
                        c_new = statep.tile([HL, BS], F32, tag="c")
                        nc.vector.scalar_tensor_tensor(
                            c_new[:], u[:, 0, :], 0.5, u[:, 1, :],
                            op0=MULT, op1=ADD)
                        thc = work.tile([HL, BS], BF16, tag="thc")
                        nc.scalar.activation(thc[:], c_new[:], Tanh)
                        # om = (1+To) * mask/2 ; hm = om * thc ( = sig(o)*mask*tanh(c))
                        om = work.tile([HL, BS], BF16, tag="om")
                        nc.vector.scalar_tensor_tensor(
                            om[:], To, 1.0, mh[:, (tl + 1) * BS:(tl + 2) * BS],
                            op0=ADD, op1=MULT)
                        hm = work.tile([HL, BS], BF16, tag="hm")
                        nc.vector.tensor_tensor(hm[:], om[:], thc[:], op=MULT)
                        # hp = (1+To)*thc = 2*h  (head lhsT; Wmean pre-halved)
                        hp = work.tile([HL, BS], BF16, tag="hp")
                        nc.vector.scalar_tensor_tensor(
                            hp[:], To, 1.0, thc[:], op0=ADD, op1=MULT)
                        if hp_ctx:
                            hp_ctx.__exit__(None, None, None)
                            hp_ctx = None
                        # head matmul: mean/std accumulated per 64-step epoch
                        tmod = t % 64
                        if tmod == 0:
                            mean_ps = mean_psp.tile([BS, 64, ACT_D], F32,
                                                    tag="mean")
                        nc.tensor.matmul(
                            mean_ps[:, tmod, :], hp[:], wms_sb[:],
                            start=True, stop=True)
                        if tmod == 63:
                            e = t // 64
                            diff = work.tile([BS, 64, ACT_D], F32, tag="diff")
                            nc.vector.tensor_tensor(
                                diff[:], az_sb[:, e * 64:(e + 1) * 64, :],
                                mean_ps[:], op=SUB)
                            sq = work.tile([BS, 64, ACT_D], BF16, tag="sq")
                            nc.vector.tensor_tensor(
                                sq[:], diff[:], diff[:], op=MULT)
                            nc.vector.tensor_reduce(
                                lp_sb[:, e * 64:(e + 1) * 64], sq[:],
                                axis=mybir.AxisListType.X, op=ADD)
                        if hp_ctx: hp_ctx.__exit__(None, None, None)
                        c_prev = c_new
                        hm_prev = hm
                        if t == nchunk * TC - 1:
                            Tt_last, thc_last = Tt, thc

            # ---- final outputs ----
            # h_T = 0.5*(1+To)*tanh(c_T), recomputed in f32 off the last step
            thc32 = work.tile([HL, BS], F32, tag="thc32")
            nc.scalar.activation(thc32[:], c_prev[:], Tanh)
            hf1 = work.tile([HL, BS], F32, tag="hf1")
            nc.vector.scalar_tensor_tensor(
                hf1[:], Tt_last[:, 3, :], 1.0, thc32[:], op0=ADD, op1=MULT)
            hf = work.tile([HL, BS], F32, tag="hf")
            nc.vector.tensor_scalar_mul(hf[:], hf1[:], 0.5)
            nc.sync.dma_start(h_o[:], hf[:])
            nc.sync.dma_start(c_o[:], c_prev[:])
            ncol = max(64, (nchunk * TC // 64) * 64)
            nc.sync.dma_start(lp_o[:, 0:ncol], lp_sb[:, 0:ncol])

    nc.finalize()
    return nc


def get_nc(with_bias: bool, nchunk: int = NCHUNK):
    key = (bool(with_bias), nchunk)
    if key not in _BUILD_CACHE:
        _BUILD_CACHE[key] = _build(with_bias, nchunk)
    return _BUILD_CACHE[key]


def _prep_core_inputs(inputs):
    """Host-side layout prep. Returns (in_maps, host_ctx)."""
    state = np.asarray(inputs["state"], np.float32)
    done = np.asarray(inputs["done"], np.float32)
    h0 = np.asarray(inputs["h0"], np.float32)
    c0 = np.asarray(inputs["c0"], np.float32)
    action = np.asarray(inputs["action"], np.float32)
    z = np.asarray(inputs["z"], np.float32)
    W1 = np.asarray(inputs["W1"], np.float32)
    W2 = np.asarray(inputs["W2"], np.float32)
    Wih = np.asarray(inputs["Wih"], np.float32)
    Whh = np.asarray(inputs["Whh"], np.float32)
    bih = np.asarray(inputs["bih"], np.float32)
    bhh = np.asarray(inputs["bhh"], np.float32)
    Wmean = np.asarray(inputs["Wmean"], np.float32)
    bmean = np.asarray(inputs["bmean"], np.float32)
    logstd = np.asarray(inputs["logstd"], np.float32)

    std = np.exp(logstd[0])                      # [8]
    perm = [0, 1, 2, 3]                          # torch order kept: (i,f,g,o)
    gscale = np.array([1.0, 1.0, 2.0, 1.0], np.float32)[:, None]

    wih_r = Wih.reshape(4, HL, H2)[perm] * gscale[:, :, None]
    whh_r = Whh.reshape(4, HL, HL)[perm] * gscale[:, :, None]
    bias_r = ((bih + bhh).reshape(4, HL)[perm] * gscale).reshape(1, 4 * HL)
    with_bias = bool(np.any(bias_r != 0.0))

    wiht = np.ascontiguousarray(wih_r.reshape(4 * HL, H2).T).astype(NPBF16)
    whht = np.ascontiguousarray(whh_r.reshape(4 * HL, HL).T).astype(NPBF16)
    biasr = bias_r.astype(NPBF16)
    w1t = np.ascontiguousarray(W1.T).astype(NPBF16)         # [OBS, H1]
    w2t = np.ascontiguousarray(W2.T).astype(NPBF16)         # [H1, H2]
    wmsh = np.ascontiguousarray((Wmean / (2.0 * std[:, None])).T).astype(NPBF16)

    st = state.reshape(T, B, OBS)
    dn = done.reshape(T, B)
    ac = action.reshape(T, B, ACT_D)
    zz = z.reshape(T, B, ACT_D)

    in_maps = []
    for core in range(NCORES):
        sl = slice(core * BS, (core + 1) * BS)
        stateT = np.ascontiguousarray(
            st[:, sl, :].transpose(2, 0, 1).reshape(OBS, ROWS)).astype(NPBF16)
        mrow = 0.5 * (1.0 - dn[:, sl]).reshape(ROWS)
        mrow = np.concatenate([mrow, np.full(BS, 0.5, np.float32)])
        mhalf = np.ascontiguousarray(
            np.broadcast_to(mrow[None, :], (128, ROWS + BS))).astype(NPBF16)
        az = (ac[:, sl, :] - zz[:, sl, :] - bmean[None, None, :]) / std
        az2s = np.ascontiguousarray(
            az.transpose(1, 0, 2).reshape(BS, T * ACT_D)).astype(np.float32)
        in_maps.append({
            "stateT": stateT, "mhalf": mhalf, "az2s": az2s,
            "w1t": w1t, "w2t": w2t, "wiht": wiht, "whht": whht,
            "biasr": biasr, "wmsh": wmsh,
            "h0T": np.ascontiguousarray(h0[0, sl, :].T).astype(np.float32),
            "c0T": np.ascontiguousarray(c0[0, sl, :].T).astype(np.float32),
        })
    host_ctx = dict(action=inputs["action"], logstd=logstd, with_bias=with_bias)
    return in_maps, host_ctx


def _assemble(results, host_ctx):
    logstd = host_ctx["logstd"]
    LOG2PI = math.log(2.0 * math.pi)
    const = float(np.sum(logstd) + 0.5 * ACT_D * LOG2PI)
    lp = np.empty((T, B), np.float32)
    hT = np.empty((1, B, HL), np.float32)
    cT = np.empty((1, B, HL), np.float32)
    for core in range(NCORES):
        sl = slice(core * BS, (core + 1) * BS)
        r = results[core]
        lp[:, sl] = r["lp"].T
        hT[0, sl, :] = r["h_o"].T
        cT[0, sl, :] = r["c_o"].T
    logprob = (-0.5 * lp.reshape(T * B)) - const
    ent_row = float(np.sum(0.5 + 0.5 * LOG2PI + logstd))
    entropy = np.full(T * B, ent_row, np.float32)
    action = np.asarray(host_ctx["action"], np.float32)
    return action, logprob.astype(np.float32), entropy, hT, cT


def _run_device(in_maps, with_bias):
    nc = get_nc(with_bias)
    res = run_bass_kernel_spmd(nc, in_maps, core_ids=list(range(NCORES)))
    return res.results


def _worker_main(in_pkl, out_pkl):
    import pickle
    with open(in_pkl, "rb") as f:
        d = pickle.load(f)
    results = _run_device(d["in_maps"], d["with_bias"])
    with open(out_pkl, "wb") as f:
        pickle.dump(results, f)


def kernel(**inputs):
    """Full-input entry point. Tries an in-process device run first; on an
    (intermittent) NRT fault the accelerator is unrecoverable for the owning
    process, so it falls back to fresh-subprocess retries."""
    import os
    import pickle
    import subprocess
    import sys
    import tempfile
    import time

    in_maps, host_ctx = _prep_core_inputs(inputs)
    with tempfile.TemporaryDirectory() as td:
        in_pkl = os.path.join(td, "in.pkl")
        out_pkl = os.path.join(td, "out.pkl")
        with open(in_pkl, "wb") as f:
            pickle.dump({"in_maps": in_maps,
                         "with_bias": host_ctx["with_bias"]}, f)
        # fast path: run in-process (build is cached for repeat calls)
        try:
            results = _run_device(in_maps, host_ctx["with_bias"])
            return _assemble(results, host_ctx)
        except Exception as e:
            last_err = e
        # device fault: the owning process can't recover the accelerator;
        # retry in fresh subprocesses after letting the pool reset
        for attempt in range(2):
            time.sleep(75)
            try:
                proc = subprocess.run(
                    [sys.executable, os.path.abspath(__file__),
                     "--worker", in_pkl, out_pkl],
                    timeout=1800, capture_output=True, text=True)
                if proc.returncode == 0 and os.path.exists(out_pkl):
                    with open(out_pkl, "rb") as f:
                        results = pickle.load(f)
                    return _assemble(results, host_ctx)
                last_err = RuntimeError(
                    f"worker rc={proc.returncode}: {proc.stderr[-2000:]}")
            except Exception as e:
                last_err = e
        raise last_err


if __name__ == "__main__":
    import sys
    if len(sys.argv) == 4 and sys.argv[1] == "--worker":
        _worker_main(sys.argv[2], sys.argv[3])


# revision 34
# speedup vs baseline: 1.0194x; 1.0067x over previous
"""Trainium2 Bass kernel for the RPO actor (MLP encoder -> masked LSTM -> Gaussian head).

Sharding: data-parallel over the env dim B (1024 envs -> 128 per NeuronCore),
weights replicated, no collectives. Host does all layout work (transposes,
gate reorder, mask broadcast); device runs feature-major ("transposed")
matmuls + a tanh-only LSTM cell:
    sigmoid(x) = (1 + tanh(x/2)) / 2
so the four gate activations collapse into ONE tanh(0.5*gates) op per step
(g-gate weights are pre-doubled on the host), and the sigmoid affines are
folded into fused scalar_tensor_tensor ops / host-side constant scalings.
"""

import math
import numpy as np
import ml_dtypes
from contextlib import ExitStack

import concourse.bass as bass
import concourse.tile as tile
import concourse.mybir as mybir
from concourse import bacc
from concourse.bass_utils import run_bass_kernel_spmd

OBS, ACT_D = 128, 8
T, B = 256, 1024
H1, H2, HL = 512, 256, 128
NCORES = 8
BS = B // NCORES          # 128 envs per core
ROWS = T * BS             # 32768 rows per core
TC = 64                   # timesteps per chunk
NCHUNK = T // TC
G = 2                     # steps per x-matmul group (must divide TC)
PRIO = 20000               # high-priority offset for the LSTM chain (0=off)

BF16 = mybir.dt.bfloat16
F32 = mybir.dt.float32
NPBF16 = ml_dtypes.bfloat16
Tanh = mybir.ActivationFunctionType.Tanh
ADD = mybir.AluOpType.add
MULT = mybir.AluOpType.mult
SUB = mybir.AluOpType.subtract

_BUILD_CACHE = {}


def _build(with_bias: bool, nchunk: int = NCHUNK):
    nc = bacc.Bacc("TRN2", target_bir_lowering=False, debug=False,
                   num_devices=NCORES)
    P = lambda n, s, d, o=False: nc.declare_dram_parameter(n, s, d, isOutput=o)
    stateT = P("stateT", [OBS, ROWS], BF16)
    mhalf = P("mhalf", [128, (T + 1) * BS], BF16)   # 0.5*(1-done), bcast over partitions
    az2s = P("az2s", [BS, T * ACT_D], F32)          # (action - z - bmean)/std, env-major
    w1t = P("w1t", [OBS, H1], BF16)
    w2t = P("w2t", [H1, H2], BF16)
    wiht = P("wiht", [H2, 4 * HL], BF16)            # gate cols (i,f,o,2g)
    whht = P("whht", [HL, 4 * HL], BF16)
    biasr = P("biasr", [1, 4 * HL], BF16)           # (bih+bhh) reordered, g doubled
    wmsh = P("wmsh", [HL, ACT_D], BF16)             # (Wmean/std).T / 2
    h0T = P("h0T", [HL, BS], F32)
    c0T = P("c0T", [HL, BS], F32)
    lp_o = P("lp", [BS, T], F32, True)              # sum-of-squares, env-major
    h_o = P("h_o", [HL, BS], F32, True)
    c_o = P("c_o", [HL, BS], F32, True)

    with tile.TileContext(nc) as tc:
        with ExitStack() as ctx:
            const = ctx.enter_context(tc.tile_pool(name="const", bufs=1))
            stream = ctx.enter_context(tc.tile_pool(name="stream", bufs=2))
            xpool = ctx.enter_context(tc.tile_pool(name="xpool", bufs=2))
            h1pool = ctx.enter_context(tc.tile_pool(name="h1pool", bufs=3))
            work = ctx.enter_context(tc.tile_pool(name="work", bufs=4))
            statep = ctx.enter_context(tc.tile_pool(name="statep", bufs=3))
            mlp_ps = ctx.enter_context(
                tc.tile_pool(name="mlp_ps", bufs=1, space="PSUM"))
            gate_ps = ctx.enter_context(
                tc.tile_pool(name="gate_ps", bufs=2, space="PSUM"))
            mean_psp = ctx.enter_context(
                tc.tile_pool(name="mean_ps", bufs=2, space="PSUM"))

            # ---- load constants ----
            w1_sb = const.tile([OBS, H1], BF16, tag="w1")
            nc.sync.dma_start(w1_sb[:], w1t[:])
            w2_sb = [const.tile([128, H2], BF16, tag=f"w2_{k}", name=f"w2_{k}")
                     for k in range(4)]
            for k in range(4):
                nc.sync.dma_start(w2_sb[k][:], w2t[k * 128:(k + 1) * 128, :])
            wih_sb = [const.tile([128, 4 * HL], BF16, tag=f"wih_{k}", name=f"wih_{k}")
                      for k in range(2)]
            for k in range(2):
                nc.sync.dma_start(wih_sb[k][:], wiht[k * 128:(k + 1) * 128, :])
            whh_sb = const.tile([HL, 4 * HL], BF16, tag="whh")
            nc.sync.dma_start(whh_sb[:], whht[:])
            wms_sb = const.tile([HL, ACT_D], BF16, tag="wms")
            az_sb = const.tile([BS, T, ACT_D], F32, tag="az")
            h0_sb = const.tile([HL, BS], F32, tag="h0")
            c0_sb = const.tile([HL, BS], F32, tag="c0")
            lp_sb = const.tile([BS, T], F32, tag="lp")
            if with_bias:
                bias_sb = const.tile([1, 4 * HL], BF16, tag="bias")
                nc.sync.dma_start(bias_sb[:], biasr[:])
                ones_sb = const.tile([1, G * BS], BF16, tag="ones")
                nc.vector.memset(ones_sb[:], 1.0)

            c_prev = c0_sb
            hm_prev = None  # set after mask of chunk 0 is loaded
            mean_ps = None
            Tt_last = None
            thc_last = None

            for c in range(nchunk):
                rl0 = c * TC * BS  # first row (in ROWS) of this chunk
                # ---- stream in state + mask ----
                # chunk 0: split the loads so the first MLP tile's state and
                # the first steps' masks land before the bulk transfer
                stT = stream.tile([OBS, TC * BS], BF16, tag="stT")
                mh = stream.tile([128, (TC + 1) * BS], BF16, tag="mh")
                if c == 0:
                    nc.sync.dma_start(stT[:, 0:1024], stateT[:, rl0:rl0 + 1024])
                    nc.sync.dma_start(mh[:, 0:4 * BS],
                                      mhalf[:, rl0:rl0 + 4 * BS])
                    nc.sync.dma_start(stT[:, 1024:TC * BS],
                                      stateT[:, rl0 + 1024:rl0 + TC * BS])
                    nc.sync.dma_start(mh[:, 4 * BS:(TC + 1) * BS],
                                      mhalf[:, rl0 + 4 * BS:rl0 + (TC + 1) * BS])
                else:
                    nc.sync.dma_start(stT[:], stateT[:, rl0:rl0 + TC * BS])
                    nc.sync.dma_start(mh[:], mhalf[:, rl0:rl0 + (TC + 1) * BS])

                if c == 0:
                    # deferred const loads (not needed for the first MLP tiles)
                    nc.sync.dma_start(h0_sb[:], h0T[:])
                    nc.sync.dma_start(c0_sb[:], c0T[:])
                    nc.sync.dma_start(wms_sb[:], wmsh[:])
                    nc.sync.dma_start(az_sb[:], az2s[:])
                    # hm_0 = (2*h0) * mhalf[0]  (= h0 * mask)
                    hm0 = work.tile([HL, BS], BF16, tag="hm")
                    nc.vector.scalar_tensor_tensor(
                        hm0[:], h0_sb[:], 2.0, mh[:, 0:BS], op0=MULT, op1=MULT)
                    hm_prev = hm0

                # ---- MLP, streamed per 1024-row tile (layer1 then layer2) ----
                xT = [xpool.tile([128, TC * BS], BF16, tag=f"xt{jj}", name=f"xt{jj}")
                      for jj in range(2)]
                NQ = TC * BS // 1024
                for q in range(NQ):
                    h1q = h1pool.tile([128, 4, 1024], BF16, tag="h1q")
                    for j in range(4):
                        ps = mlp_ps.tile([128, 1024], F32, tag="mlp")
                        for s in range(2):
                            nc.tensor.matmul(
                                ps[:, s * 512:(s + 1) * 512],
                                w1_sb[:, j * 128:(j + 1) * 128],
                                stT[:, q * 1024 + s * 512:q * 1024 + (s + 1) * 512],
                                start=True, stop=True)
                        for s in range(2):
                            nc.scalar.activation(
                                h1q[:, j, s * 512:(s + 1) * 512],
                                ps[:, s * 512:(s + 1) * 512], Tanh)
                    for jj in range(2):
                        ps = mlp_ps.tile([128, 1024], F32, tag="mlp")
                        for s in range(2):
                            for k in range(4):
                                nc.tensor.matmul(
                                    ps[:, s * 512:(s + 1) * 512],
                                    w2_sb[k][:, jj * 128:(jj + 1) * 128],
                                    h1q[:, k, s * 512:(s + 1) * 512],
                                    start=(k == 0), stop=(k == 3))
                        for s in range(2):
                            nc.scalar.activation(
                                xT[jj][:, q * 1024 + s * 512:q * 1024 + (s + 1) * 512],
                                ps[:, s * 512:(s + 1) * 512], Tanh)

                # ---- LSTM scan over this chunk ----
                for tl0 in range(0, TC, G):
                    # gates psum: [feat, gate j, (dt, b)] ; x-part batched over G steps
                    # One accumulation group per PSUM bank: start=True only on
                    # the first matmul touching the bank (it clears has_written
                    # for the WHOLE bank), stop=True on the bank's last matmul
                    # (the final recurrent matmul below). With G=2, gates (i,f)
                    # share bank 0 and (o,g) share bank 1.
                    ps = gate_ps.tile([128, 4, G * BS], F32, tag="gates")
                    for j in range(4):
                        for k in range(2):
                            nc.tensor.matmul(
                                ps[:, j, :],
                                wih_sb[k][:, j * 128:(j + 1) * 128],
                                xT[k][:, tl0 * BS:(tl0 + G) * BS],
                                start=(k == 0 and j % 2 == 0), stop=False,
                                skip_group_check=True)
                        if with_bias:
                            nc.tensor.matmul(
                                ps[:, j, :],
                                bias_sb[:, j * 128:(j + 1) * 128],
                                ones_sb[:],
                                start=False, stop=False,
                                skip_group_check=True)
                    for dt in range(G):
                        tl = tl0 + dt
                        t = c * TC + tl
                        hp_ctx = tc.high_priority(offset=PRIO) if PRIO else None
                        if hp_ctx: hp_ctx.__enter__()
                        # Tt layout: [i, f, g, o, cm] along dim1. cm (the
                        # masked cell, bf16) rides in slot 4 so one fused stt
                        # computes both (1+Ti)*Tg and (1+Tf)*cm via the
                        # strided in1 AP (g at slot 2, cm at slot 4).
                        Tt = work.tile([128, 5, BS], BF16, tag="Tt")
                        nc.vector.tensor_tensor(
                            Tt[:, 4, :], c_prev[:], mh[:, tl * BS:(tl + 1) * BS],
                            op=MULT)
                        # recurrent matmuls
                        for j in range(4):
                            nc.tensor.matmul(
                                ps[:, j, dt * BS:(dt + 1) * BS],
                                whh_sb[:, j * 128:(j + 1) * 128],
                                hm_prev[:],
                                start=False,
                                stop=(dt == G - 1 and j % 2 == 1),
                                skip_group_check=True)
                        # one tanh for all gates: T = tanh(0.5 * gates)
                        nc.scalar.activation(
                            Tt[:, 0:4, :], ps[:, 0:4, dt * BS:(dt + 1) * BS],
                            Tanh, scale=0.5)
                        Ti, Tf, Tg, To = (Tt[:, 0, :], Tt[:, 1, :],
                                          Tt[:, 2, :], Tt[:, 3, :])
                        # u0 = (1+Ti)*Tg ; u1 = (1+Tf)*cm  (one fused stt)
                        u = work.tile([128, 2, BS], BF16, tag="u")
                        nc.vector.scalar_tensor_tensor(
                            u[:], Tt[:, 0:2, :], 1.0, Tt[:, 2:5:2, :],
                            op0=ADD, op1=MULT)
                        # c_new = u1 + 0.5*u0
                        c_new = statep.tile([HL, BS], F32, tag="c")
                        nc.vector.scalar_tensor_tensor(
                            c_new[:], u[:, 0, :], 0.5, u[:, 1, :],
                            op0=MULT, op1=ADD)
                        thc = work.tile([HL, BS], BF16, tag="thc")
                        nc.scalar.activation(thc[:], c_new[:], Tanh)
                        # om = (1+To) * mask/2 ; hm = om * thc ( = sig(o)*mask*tanh(c))
                        om = work.tile([HL, BS], BF16, tag="om")
                        nc.vector.scalar_tensor_tensor(
                            om[:], To, 1.0, mh[:, (tl + 1) * BS:(tl + 2) * BS],
                            op0=ADD, op1=MULT)
                        hm = work.tile([HL, BS], BF16, tag="hm")
                        nc.vector.tensor_tensor(hm[:], om[:], thc[:], op=MULT)
                        # hp = (1+To)*thc = 2*h  (head lhsT; Wmean pre-halved)
                        hp = work.tile([HL, BS], BF16, tag="hp")
                        nc.vector.scalar_tensor_tensor(
                            hp[:], To, 1.0, thc[:], op0=ADD, op1=MULT)
                        if hp_ctx:
                            hp_ctx.__exit__(None, None, None)
                            hp_ctx = None
                        # head matmul: mean/std accumulated per 64-step epoch
                        tmod = t % 64
                        if tmod == 0:
                            mean_ps = mean_psp.tile([BS, 64, ACT_D], F32,
                                                    tag="mean")
                        nc.tensor.matmul(
                            mean_ps[:, tmod, :], hp[:], wms_sb[:],
                            start=True, stop=True)
                        if tmod == 63:
                            e = t // 64
                            diff = work.tile([BS, 64, ACT_D], F32, tag="diff")
                            nc.vector.tensor_tensor(
                                diff[:], az_sb[:, e * 64:(e + 1) * 64, :],
                                mean_ps[:], op=SUB)
                            sq = work.tile([BS, 64, ACT_D], BF16, tag="sq")
                            nc.vector.tensor_tensor(
                                sq[:], diff[:], diff[:], op=MULT)
                            nc.vector.tensor_reduce(
                                lp_sb[:, e * 64:(e + 1) * 64], sq[:],
                                axis=mybir.AxisListType.X, op=ADD)
                        if hp_ctx: hp_ctx.__exit__(None, None, None)
                        c_prev = c_new
                        hm_prev = hm
                        if t == nchunk * TC - 1:
                            Tt_last, thc_last = Tt, thc

            # ---- final outputs ----
            # h_T = 0.5*(1+To)*tanh(c_T), recomputed in f32 off the last step
            thc32 = work.tile([HL, BS], F32, tag="thc32")
            nc.scalar.activation(thc32[:], c_prev[:], Tanh)
            hf1 = work.tile([HL, BS], F32, tag="hf1")
            nc.vector.scalar_tensor_tensor(
                hf1[:], Tt_last[:, 3, :], 1.0, thc32[:], op0=ADD, op1=MULT)
            hf = work.tile([HL, BS], F32, tag="hf")
            nc.vector.tensor_scalar_mul(hf[:], hf1[:], 0.5)
            nc.sync.dma_start(h_o[:], hf[:])
            nc.sync.dma_start(c_o[:], c_prev[:])
            ncol = max(64, (nchunk * TC // 64) * 64)
            nc.sync.dma_start(lp_o[:, 0:ncol], lp_sb[:, 0:ncol])

    nc.finalize()
    return nc


def get_nc(with_bias: bool, nchunk: int = NCHUNK):
    key = (bool(with_bias), nchunk)
    if key not in _BUILD_CACHE:
        _BUILD_CACHE[key] = _build(with_bias, nchunk)
    return _BUILD_CACHE[key]


def _prep_core_inputs(inputs):
    """Host-side layout prep. Returns (in_maps, host_ctx)."""
    state = np.asarray(inputs["state"], np.float32)
    done = np.asarray(inputs["done"], np.float32)
    h0 = np.asarray(inputs["h0"], np.float32)
    c0 = np.asarray(inputs["c0"], np.float32)
    action = np.asarray(inputs["action"], np.float32)
    z = np.asarray(inputs["z"], np.float32)
    W1 = np.asarray(inputs["W1"], np.float32)
    W2 = np.asarray(inputs["W2"], np.float32)
    Wih = np.asarray(inputs["Wih"], np.float32)
    Whh = np.asarray(inputs["Whh"], np.float32)
    bih = np.asarray(inputs["bih"], np.float32)
    bhh = np.asarray(inputs["bhh"], np.float32)
    Wmean = np.asarray(inputs["Wmean"], np.float32)
    bmean = np.asarray(inputs["bmean"], np.float32)
    logstd = np.asarray(inputs["logstd"], np.float32)

    std = np.exp(logstd[0])                      # [8]
    perm = [0, 1, 2, 3]                          # torch order kept: (i,f,g,o)
    gscale = np.array([1.0, 1.0, 2.0, 1.0], np.float32)[:, None]

    wih_r = Wih.reshape(4, HL, H2)[perm] * gscale[:, :, None]
    whh_r = Whh.reshape(4, HL, HL)[perm] * gscale[:, :, None]
    bias_r = ((bih + bhh).reshape(4, HL)[perm] * gscale).reshape(1, 4 * HL)
    with_bias = bool(np.any(bias_r != 0.0))

    wiht = np.ascontiguousarray(wih_r.reshape(4 * HL, H2).T).astype(NPBF16)
    whht = np.ascontiguousarray(whh_r.reshape(4 * HL, HL).T).astype(NPBF16)
    biasr = bias_r.astype(NPBF16)
    w1t = np.ascontiguousarray(W1.T).astype(NPBF16)         # [OBS, H1]
    w2t = np.ascontiguousarray(W2.T).astype(NPBF16)         # [H1, H2]
    wmsh = np.ascontiguousarray((Wmean / (2.0 * std[:, None])).T).astype(NPBF16)

    st = state.reshape(T, B, OBS)
    dn = done.reshape(T, B)
    ac = action.reshape(T, B, ACT_D)
    zz = z.reshape(T, B, ACT_D)

    in_maps = []
    for core in range(NCORES):
        sl = slice(core * BS, (core + 1) * BS)
        stateT = np.ascontiguousarray(
            st[:, sl, :].transpose(2, 0, 1).reshape(OBS, ROWS)).astype(NPBF16)
        mrow = 0.5 * (1.0 - dn[:, sl]).reshape(ROWS)
        mrow = np.concatenate([mrow, np.full(BS, 0.5, np.float32)])
        mhalf = np.ascontiguousarray(
            np.broadcast_to(mrow[None, :], (128, ROWS + BS))).astype(NPBF16)
        az = (ac[:, sl, :] - zz[:, sl, :] - bmean[None, None, :]) / std
        az2s = np.ascontiguousarray(
            az.transpose(1, 0, 2).reshape(BS, T * ACT_D)).astype(np.float32)
        in_maps.append({
            "stateT": stateT, "mhalf": mhalf, "az2s": az2s,
            "w1t": w1t, "w2t": w2t, "wiht": wiht, "whht": whht,
            "biasr": biasr, "wmsh": wmsh,
            "h0T": np.ascontiguousarray(h0[0, sl, :].T).astype(np.float32),
            "c0T": np.ascontiguousarray(c0[0, sl, :].T).astype(np.float32),
        })
    host_ctx = dict(action=inputs["action"], logstd=logstd, with_bias=with_bias)
    return in_maps, host_ctx


def _assemble(results, host_ctx):
    logstd = host_ctx["logstd"]
    LOG2PI = math.log(2.0 * math.pi)
    const = float(np.sum(logstd) + 0.5 * ACT_D * LOG2PI)
    lp = np.empty((T, B), np.float32)
    hT = np.empty((1, B, HL), np.float32)
    cT = np.empty((1, B, HL), np.float32)
    for core in range(NCORES):
        sl = slice(core * BS, (core + 1) * BS)
        r = results[core]
        lp[:, sl] = r["lp"].T
        hT[0, sl, :] = r["h_o"].T
        cT[0, sl, :] = r["c_o"].T
    logprob = (-0.5 * lp.reshape(T * B)) - const
    ent_row = float(np.sum(0.5 + 0.5 * LOG2PI + logstd))
    entropy = np.full(T * B, ent_row, np.float32)
    action = np.asarray(host_ctx["action"], np.float32)
    return action, logprob.astype(np.float32), entropy, hT, cT


def _run_device(in_maps, with_bias):
    nc = get_nc(with_bias)
    res = run_bass_kernel_spmd(nc, in_maps, core_ids=list(range(NCORES)))
    return res.results


def _worker_main(in_pkl, out_pkl):
    import pickle
    with open(in_pkl, "rb") as f:
        d = pickle.load(f)
    results = _run_device(d["in_maps"], d["with_bias"])
    with open(out_pkl, "wb") as f:
        pickle.dump(results, f)


def kernel(**inputs):
    """Full-input entry point. Tries an in-process device run first; on an
    (intermittent) NRT fault the accelerator is unrecoverable for the owning
    process, so it falls back to fresh-subprocess retries."""
    import os
    import pickle
    import subprocess
    import sys
    import tempfile
    import time

    in_maps, host_ctx = _prep_core_inputs(inputs)
    with tempfile.TemporaryDirectory() as td:
        in_pkl = os.path.join(td, "in.pkl")
        out_pkl = os.path.join(td, "out.pkl")
        with open(in_pkl, "wb") as f:
            pickle.dump({"in_maps": in_maps,
                         "with_bias": host_ctx["with_bias"]}, f)
        # fast path: run in-process (build is cached for repeat calls)
        try:
            results = _run_device(in_maps, host_ctx["with_bias"])
            return _assemble(results, host_ctx)
        except Exception as e:
            last_err = e
        # device fault: the owning process can't recover the accelerator;
        # retry in fresh subprocesses after letting the pool reset
        for attempt in range(2):
            time.sleep(75)
            try:
                proc = subprocess.run(
                    [sys.executable, os.path.abspath(__file__),
                     "--worker", in_pkl, out_pkl],
                    timeout=1800, capture_output=True, text=True)
                if proc.returncode == 0 and os.path.exists(out_pkl):
                    with open(out_pkl, "rb") as f:
                        results = pickle.load(f)
                    return _assemble(results, host_ctx)
                last_err = RuntimeError(
                    f"worker rc={proc.returncode}: {proc.stderr[-2000:]}")
            except Exception as e:
                last_err = e
        raise last_err


if __name__ == "__main__":
    import sys
    if len(sys.argv) == 4 and sys.argv[1] == "--worker":
        _worker_main(sys.argv[2], sys.argv[3])


# revision 35
# speedup vs baseline: 1.0235x; 1.0040x over previous
"""Trainium2 Bass kernel for the RPO actor (MLP encoder -> masked LSTM -> Gaussian head).

Sharding: data-parallel over the env dim B (1024 envs -> 128 per NeuronCore),
weights replicated, no collectives. Host does all layout work (transposes,
gate reorder, mask broadcast); device runs feature-major ("transposed")
matmuls + a tanh-only LSTM cell:
    sigmoid(x) = (1 + tanh(x/2)) / 2
so the four gate activations collapse into ONE tanh(0.5*gates) op per step
(g-gate weights are pre-doubled on the host), and the sigmoid affines are
folded into fused scalar_tensor_tensor ops / host-side constant scalings.
"""

import math
import numpy as np
import ml_dtypes
from contextlib import ExitStack

import concourse.bass as bass
import concourse.tile as tile
import concourse.mybir as mybir
from concourse import bacc
from concourse.bass_utils import run_bass_kernel_spmd

OBS, ACT_D = 128, 8
T, B = 256, 1024
H1, H2, HL = 512, 256, 128
NCORES = 8
BS = B // NCORES          # 128 envs per core
ROWS = T * BS             # 32768 rows per core
TC = 64                   # timesteps per chunk
NCHUNK = T // TC
G = 2                     # steps per x-matmul group (must divide TC)
PRIO = 20000               # high-priority offset for the LSTM chain (0=off)

BF16 = mybir.dt.bfloat16
F32 = mybir.dt.float32
NPBF16 = ml_dtypes.bfloat16
Tanh = mybir.ActivationFunctionType.Tanh
ADD = mybir.AluOpType.add
MULT = mybir.AluOpType.mult
SUB = mybir.AluOpType.subtract

_BUILD_CACHE = {}


def _build(with_bias: bool, nchunk: int = NCHUNK):
    nc = bacc.Bacc("TRN2", target_bir_lowering=False, debug=False,
                   num_devices=NCORES)
    P = lambda n, s, d, o=False: nc.declare_dram_parameter(n, s, d, isOutput=o)
    stateT = P("stateT", [OBS, ROWS], BF16)
    mhalf = P("mhalf", [128, (T + 1) * BS], BF16)   # 0.5*(1-done), bcast over partitions
    az2s = P("az2s", [BS, T * ACT_D], F32)          # (action - z - bmean)/std, env-major
    w1t = P("w1t", [OBS, H1], BF16)
    w2t = P("w2t", [H1, H2], BF16)
    wiht = P("wiht", [H2, 4 * HL], BF16)            # gate cols (i,f,o,2g)
    whht = P("whht", [HL, 4 * HL], BF16)
    biasr = P("biasr", [1, 4 * HL], BF16)           # (bih+bhh) reordered, g doubled
    wmsh = P("wmsh", [HL, ACT_D], BF16)             # (Wmean/std).T / 2
    h0T = P("h0T", [HL, BS], F32)
    c0T = P("c0T", [HL, BS], F32)
    lp_o = P("lp", [BS, T], F32, True)              # sum-of-squares, env-major
    h_o = P("h_o", [HL, BS], F32, True)
    c_o = P("c_o", [HL, BS], F32, True)

    with tile.TileContext(nc) as tc:
        with ExitStack() as ctx:
            const = ctx.enter_context(tc.tile_pool(name="const", bufs=1))
            stream = ctx.enter_context(tc.tile_pool(name="stream", bufs=2))
            xpool = ctx.enter_context(tc.tile_pool(name="xpool", bufs=2))
            h1pool = ctx.enter_context(tc.tile_pool(name="h1pool", bufs=3))
            work = ctx.enter_context(tc.tile_pool(name="work", bufs=4))
            statep = ctx.enter_context(tc.tile_pool(name="statep", bufs=3))
            mlp_ps = ctx.enter_context(
                tc.tile_pool(name="mlp_ps", bufs=1, space="PSUM"))
            gate_ps = ctx.enter_context(
                tc.tile_pool(name="gate_ps", bufs=2, space="PSUM"))
            mean_psp = ctx.enter_context(
                tc.tile_pool(name="mean_ps", bufs=2, space="PSUM"))

            # ---- load constants ----
            w1_sb = const.tile([OBS, H1], BF16, tag="w1")
            nc.sync.dma_start(w1_sb[:], w1t[:])
            w2_sb = [const.tile([128, H2], BF16, tag=f"w2_{k}", name=f"w2_{k}")
                     for k in range(4)]
            for k in range(4):
                nc.sync.dma_start(w2_sb[k][:], w2t[k * 128:(k + 1) * 128, :])
            wih_sb = [const.tile([128, 4 * HL], BF16, tag=f"wih_{k}", name=f"wih_{k}")
                      for k in range(2)]
            for k in range(2):
                nc.sync.dma_start(wih_sb[k][:], wiht[k * 128:(k + 1) * 128, :])
            whh_sb = const.tile([HL, 4 * HL], BF16, tag="whh")
            nc.sync.dma_start(whh_sb[:], whht[:])
            wms_sb = const.tile([HL, ACT_D], BF16, tag="wms")
            az_sb = const.tile([BS, T, ACT_D], F32, tag="az")
            h0_sb = const.tile([HL, BS], F32, tag="h0")
            c0_sb = const.tile([HL, BS], F32, tag="c0")
            lp_sb = const.tile([BS, T], F32, tag="lp")
            if with_bias:
                bias_sb = const.tile([1, 4 * HL], BF16, tag="bias")
                nc.sync.dma_start(bias_sb[:], biasr[:])
                ones_sb = const.tile([1, G * BS], BF16, tag="ones")
                nc.vector.memset(ones_sb[:], 1.0)

            c_prev = c0_sb
            hm_prev = None  # set after mask of chunk 0 is loaded
            mean_ps = None
            Tt_last = None
            thc_last = None

            for c in range(nchunk):
                rl0 = c * TC * BS  # first row (in ROWS) of this chunk
                # ---- stream in state + mask ----
                # chunk 0: split the loads so the first MLP tile's state and
                # the first steps' masks land before the bulk transfer
                stT = stream.tile([OBS, TC * BS], BF16, tag="stT")
                mh = stream.tile([128, (TC + 1) * BS], BF16, tag="mh")
                if c == 0:
                    nc.sync.dma_start(stT[:, 0:1024], stateT[:, rl0:rl0 + 1024])
                    nc.sync.dma_start(mh[:, 0:4 * BS],
                                      mhalf[:, rl0:rl0 + 4 * BS])
                    nc.sync.dma_start(stT[:, 1024:TC * BS],
                                      stateT[:, rl0 + 1024:rl0 + TC * BS])
                    nc.sync.dma_start(mh[:, 4 * BS:(TC + 1) * BS],
                                      mhalf[:, rl0 + 4 * BS:rl0 + (TC + 1) * BS])
                else:
                    nc.sync.dma_start(stT[:], stateT[:, rl0:rl0 + TC * BS])
                    nc.sync.dma_start(mh[:], mhalf[:, rl0:rl0 + (TC + 1) * BS])

                if c == 0:
                    # deferred const loads (not needed for the first MLP tiles)
                    nc.sync.dma_start(h0_sb[:], h0T[:])
                    nc.sync.dma_start(c0_sb[:], c0T[:])
                    nc.sync.dma_start(wms_sb[:], wmsh[:])
                    nc.sync.dma_start(az_sb[:], az2s[:])
                    # hm_0 = (2*h0) * mhalf[0]  (= h0 * mask)
                    hm0 = work.tile([HL, BS], BF16, tag="hm")
                    nc.vector.scalar_tensor_tensor(
                        hm0[:], h0_sb[:], 2.0, mh[:, 0:BS], op0=MULT, op1=MULT)
                    hm_prev = hm0

                # ---- MLP, streamed per 1024-row tile (layer1 then layer2) ----
                xT = [xpool.tile([128, TC * BS], BF16, tag=f"xt{jj}", name=f"xt{jj}")
                      for jj in range(2)]
                NQ = TC * BS // 1024
                for q in range(NQ):
                    if c == 0 and q == 0:
                        # fast-start tile: half-width sub-tiles so the first
                        # 512 x-columns (steps 0-3) are ready ~6us sooner
                        h1q = h1pool.tile([128, 4, 1024], BF16, tag="h1q",
                                          name="h1q0")
                        for s in range(2):
                            for j in range(4):
                                ps = mlp_ps.tile([128, 512], F32, tag="mlp",
                                                 name="mlp0")
                                nc.tensor.matmul(
                                    ps[:],
                                    w1_sb[:, j * 128:(j + 1) * 128],
                                    stT[:, s * 512:(s + 1) * 512],
                                    start=True, stop=True)
                                nc.scalar.activation(
                                    h1q[:, j, s * 512:(s + 1) * 512],
                                    ps[:], Tanh)
                            for jj in range(2):
                                ps = mlp_ps.tile([128, 512], F32, tag="mlp",
                                                 name="mlp0")
                                for k in range(4):
                                    nc.tensor.matmul(
                                        ps[:],
                                        w2_sb[k][:, jj * 128:(jj + 1) * 128],
                                        h1q[:, k, s * 512:(s + 1) * 512],
                                        start=(k == 0), stop=(k == 3))
                                nc.scalar.activation(
                                    xT[jj][:, s * 512:(s + 1) * 512],
                                    ps[:], Tanh)
                        continue
                    h1q = h1pool.tile([128, 4, 1024], BF16, tag="h1q")
                    for j in range(4):
                        ps = mlp_ps.tile([128, 1024], F32, tag="mlp")
                        for s in range(2):
                            nc.tensor.matmul(
                                ps[:, s * 512:(s + 1) * 512],
                                w1_sb[:, j * 128:(j + 1) * 128],
                                stT[:, q * 1024 + s * 512:q * 1024 + (s + 1) * 512],
                                start=True, stop=True)
                        for s in range(2):
                            nc.scalar.activation(
                                h1q[:, j, s * 512:(s + 1) * 512],
                                ps[:, s * 512:(s + 1) * 512], Tanh)
                    for jj in range(2):
                        ps = mlp_ps.tile([128, 1024], F32, tag="mlp")
                        for s in range(2):
                            for k in range(4):
                                nc.tensor.matmul(
                                    ps[:, s * 512:(s + 1) * 512],
                                    w2_sb[k][:, jj * 128:(jj + 1) * 128],
                                    h1q[:, k, s * 512:(s + 1) * 512],
                                    start=(k == 0), stop=(k == 3))
                        for s in range(2):
                            nc.scalar.activation(
                                xT[jj][:, q * 1024 + s * 512:q * 1024 + (s + 1) * 512],
                                ps[:, s * 512:(s + 1) * 512], Tanh)

                # ---- LSTM scan over this chunk ----
                for tl0 in range(0, TC, G):
                    # gates psum: [feat, gate j, (dt, b)] ; x-part batched over G steps
                    # One accumulation group per PSUM bank: start=True only on
                    # the first matmul touching the bank (it clears has_written
                    # for the WHOLE bank), stop=True on the bank's last matmul
                    # (the final recurrent matmul below). With G=2, gates (i,f)
                    # share bank 0 and (o,g) share bank 1.
                    ps = gate_ps.tile([128, 4, G * BS], F32, tag="gates")
                    for j in range(4):
                        for k in range(2):
                            nc.tensor.matmul(
                                ps[:, j, :],
                                wih_sb[k][:, j * 128:(j + 1) * 128],
                                xT[k][:, tl0 * BS:(tl0 + G) * BS],
                                start=(k == 0 and j % 2 == 0), stop=False,
                                skip_group_check=True)
                        if with_bias:
                            nc.tensor.matmul(
                                ps[:, j, :],
                                bias_sb[:, j * 128:(j + 1) * 128],
                                ones_sb[:],
                                start=False, stop=False,
                                skip_group_check=True)
                    for dt in range(G):
                        tl = tl0 + dt
                        t = c * TC + tl
                        hp_ctx = tc.high_priority(offset=PRIO) if PRIO else None
                        if hp_ctx: hp_ctx.__enter__()
                        # Tt layout: [i, f, g, o, cm] along dim1. cm (the
                        # masked cell, bf16) rides in slot 4 so one fused stt
                        # computes both (1+Ti)*Tg and (1+Tf)*cm via the
                        # strided in1 AP (g at slot 2, cm at slot 4).
                        Tt = work.tile([128, 5, BS], BF16, tag="Tt")
                        nc.vector.tensor_tensor(
                            Tt[:, 4, :], c_prev[:], mh[:, tl * BS:(tl + 1) * BS],
                            op=MULT)
                        # recurrent matmuls
                        for j in range(4):
                            nc.tensor.matmul(
                                ps[:, j, dt * BS:(dt + 1) * BS],
                                whh_sb[:, j * 128:(j + 1) * 128],
                                hm_prev[:],
                                start=False,
                                stop=(dt == G - 1 and j % 2 == 1),
                                skip_group_check=True)
                        # one tanh for all gates: T = tanh(0.5 * gates)
                        nc.scalar.activation(
                            Tt[:, 0:4, :], ps[:, 0:4, dt * BS:(dt + 1) * BS],
                            Tanh, scale=0.5)
                        Ti, Tf, Tg, To = (Tt[:, 0, :], Tt[:, 1, :],
                                          Tt[:, 2, :], Tt[:, 3, :])
                        # u0 = (1+Ti)*Tg ; u1 = (1+Tf)*cm  (one fused stt)
                        u = work.tile([128, 2, BS], BF16, tag="u")
                        nc.vector.scalar_tensor_tensor(
                            u[:], Tt[:, 0:2, :], 1.0, Tt[:, 2:5:2, :],
                            op0=ADD, op1=MULT)
                        # c_new = u1 + 0.5*u0
                        c_new = statep.tile([HL, BS], F32, tag="c")
                        nc.vector.scalar_tensor_tensor(
                            c_new[:], u[:, 0, :], 0.5, u[:, 1, :],
                            op0=MULT, op1=ADD)
                        thc = work.tile([HL, BS], BF16, tag="thc")
                        nc.scalar.activation(thc[:], c_new[:], Tanh)
                        # om = (1+To) * mask/2 ; hm = om * thc ( = sig(o)*mask*tanh(c))
                        om = work.tile([HL, BS], BF16, tag="om")
                        nc.vector.scalar_tensor_tensor(
                            om[:], To, 1.0, mh[:, (tl + 1) * BS:(tl + 2) * BS],
                            op0=ADD, op1=MULT)
                        hm = work.tile([HL, BS], BF16, tag="hm")
                        nc.vector.tensor_tensor(hm[:], om[:], thc[:], op=MULT)
                        # hp = (1+To)*thc = 2*h  (head lhsT; Wmean pre-halved)
                        hp = work.tile([HL, BS], BF16, tag="hp")
                        nc.vector.scalar_tensor_tensor(
                            hp[:], To, 1.0, thc[:], op0=ADD, op1=MULT)
                        if hp_ctx:
                            hp_ctx.__exit__(None, None, None)
                            hp_ctx = None
                        # head matmul: mean/std accumulated per 64-step epoch
                        tmod = t % 64
                        if tmod == 0:
                            mean_ps = mean_psp.tile([BS, 64, ACT_D], F32,
                                                    tag="mean")
                        nc.tensor.matmul(
                            mean_ps[:, tmod, :], hp[:], wms_sb[:],
                            start=True, stop=True)
                        if tmod == 63:
                            e = t // 64
                            diff = work.tile([BS, 64, ACT_D], F32, tag="diff")
                            nc.vector.tensor_tensor(
                                diff[:], az_sb[:, e * 64:(e + 1) * 64, :],
                                mean_ps[:], op=SUB)
                            sq = work.tile([BS, 64, ACT_D], BF16, tag="sq")
                            nc.vector.tensor_tensor(
                                sq[:], diff[:], diff[:], op=MULT)
                            nc.vector.tensor_reduce(
                                lp_sb[:, e * 64:(e + 1) * 64], sq[:],
                                axis=mybir.AxisListType.X, op=ADD)
                        if hp_ctx: hp_ctx.__exit__(None, None, None)
                        c_prev = c_new
                        hm_prev = hm
                        if t == nchunk * TC - 1:
                            Tt_last, thc_last = Tt, thc

            # ---- final outputs ----
            # h_T = 0.5*(1+To)*tanh(c_T), recomputed in f32 off the last step
            thc32 = work.tile([HL, BS], F32, tag="thc32")
            nc.scalar.activation(thc32[:], c_prev[:], Tanh)
            hf1 = work.tile([HL, BS], F32, tag="hf1")
            nc.vector.scalar_tensor_tensor(
                hf1[:], Tt_last[:, 3, :], 1.0, thc32[:], op0=ADD, op1=MULT)
            hf = work.tile([HL, BS], F32, tag="hf")
            nc.vector.tensor_scalar_mul(hf[:], hf1[:], 0.5)
            nc.sync.dma_start(h_o[:], hf[:])
            nc.sync.dma_start(c_o[:], c_prev[:])
            ncol = max(64, (nchunk * TC // 64) * 64)
            nc.sync.dma_start(lp_o[:, 0:ncol], lp_sb[:, 0:ncol])

    nc.finalize()
    return nc


def get_nc(with_bias: bool, nchunk: int = NCHUNK):
    key = (bool(with_bias), nchunk)
    if key not in _BUILD_CACHE:
        _BUILD_CACHE[key] = _build(with_bias, nchunk)
    return _BUILD_CACHE[key]


def _prep_core_inputs(inputs):
    """Host-side layout prep. Returns (in_maps, host_ctx)."""
    state = np.asarray(inputs["state"], np.float32)
    done = np.asarray(inputs["done"], np.float32)
    h0 = np.asarray(inputs["h0"], np.float32)
    c0 = np.asarray(inputs["c0"], np.float32)
    action = np.asarray(inputs["action"], np.float32)
    z = np.asarray(inputs["z"], np.float32)
    W1 = np.asarray(inputs["W1"], np.float32)
    W2 = np.asarray(inputs["W2"], np.float32)
    Wih = np.asarray(inputs["Wih"], np.float32)
    Whh = np.asarray(inputs["Whh"], np.float32)
    bih = np.asarray(inputs["bih"], np.float32)
    bhh = np.asarray(inputs["bhh"], np.float32)
    Wmean = np.asarray(inputs["Wmean"], np.float32)
    bmean = np.asarray(inputs["bmean"], np.float32)
    logstd = np.asarray(inputs["logstd"], np.float32)

    std = np.exp(logstd[0])                      # [8]
    perm = [0, 1, 2, 3]                          # torch order kept: (i,f,g,o)
    gscale = np.array([1.0, 1.0, 2.0, 1.0], np.float32)[:, None]

    wih_r = Wih.reshape(4, HL, H2)[perm] * gscale[:, :, None]
    whh_r = Whh.reshape(4, HL, HL)[perm] * gscale[:, :, None]
    bias_r = ((bih + bhh).reshape(4, HL)[perm] * gscale).reshape(1, 4 * HL)
    with_bias = bool(np.any(bias_r != 0.0))

    wiht = np.ascontiguousarray(wih_r.reshape(4 * HL, H2).T).astype(NPBF16)
    whht = np.ascontiguousarray(whh_r.reshape(4 * HL, HL).T).astype(NPBF16)
    biasr = bias_r.astype(NPBF16)
    w1t = np.ascontiguousarray(W1.T).astype(NPBF16)         # [OBS, H1]
    w2t = np.ascontiguousarray(W2.T).astype(NPBF16)         # [H1, H2]
    wmsh = np.ascontiguousarray((Wmean / (2.0 * std[:, None])).T).astype(NPBF16)

    st = state.reshape(T, B, OBS)
    dn = done.reshape(T, B)
    ac = action.reshape(T, B, ACT_D)
    zz = z.reshape(T, B, ACT_D)

    in_maps = []
    for core in range(NCORES):
        sl = slice(core * BS, (core + 1) * BS)
        stateT = np.ascontiguousarray(
            st[:, sl, :].transpose(2, 0, 1).reshape(OBS, ROWS)).astype(NPBF16)
        mrow = 0.5 * (1.0 - dn[:, sl]).reshape(ROWS)
        mrow = np.concatenate([mrow, np.full(BS, 0.5, np.float32)])
        mhalf = np.ascontiguousarray(
            np.broadcast_to(mrow[None, :], (128, ROWS + BS))).astype(NPBF16)
        az = (ac[:, sl, :] - zz[:, sl, :] - bmean[None, None, :]) / std
        az2s = np.ascontiguousarray(
            az.transpose(1, 0, 2).reshape(BS, T * ACT_D)).astype(np.float32)
        in_maps.append({
            "stateT": stateT, "mhalf": mhalf, "az2s": az2s,
            "w1t": w1t, "w2t": w2t, "wiht": wiht, "whht": whht,
            "biasr": biasr, "wmsh": wmsh,
            "h0T": np.ascontiguousarray(h0[0, sl, :].T).astype(np.float32),
            "c0T": np.ascontiguousarray(c0[0, sl, :].T).astype(np.float32),
        })
    host_ctx = dict(action=inputs["action"], logstd=logstd, with_bias=with_bias)
    return in_maps, host_ctx


def _assemble(results, host_ctx):
    logstd = host_ctx["logstd"]
    LOG2PI = math.log(2.0 * math.pi)
    const = float(np.sum(logstd) + 0.5 * ACT_D * LOG2PI)
    lp = np.empty((T, B), np.float32)
    hT = np.empty((1, B, HL), np.float32)
    cT = np.empty((1, B, HL), np.float32)
    for core in range(NCORES):
        sl = slice(core * BS, (core + 1) * BS)
        r = results[core]
        lp[:, sl] = r["lp"].T
        hT[0, sl, :] = r["h_o"].T
        cT[0, sl, :] = r["c_o"].T
    logprob = (-0.5 * lp.reshape(T * B)) - const
    ent_row = float(np.sum(0.5 + 0.5 * LOG2PI + logstd))
    entropy = np.full(T * B, ent_row, np.float32)
    action = np.asarray(host_ctx["action"], np.float32)
    return action, logprob.astype(np.float32), entropy, hT, cT


def _run_device(in_maps, with_bias):
    nc = get_nc(with_bias)
    res = run_bass_kernel_spmd(nc, in_maps, core_ids=list(range(NCORES)))
    return res.results


def _worker_main(in_pkl, out_pkl):
    import pickle
    with open(in_pkl, "rb") as f:
        d = pickle.load(f)
    results = _run_device(d["in_maps"], d["with_bias"])
    with open(out_pkl, "wb") as f:
        pickle.dump(results, f)


def kernel(**inputs):
    """Full-input entry point. Tries an in-process device run first; on an
    (intermittent) NRT fault the accelerator is unrecoverable for the owning
    process, so it falls back to fresh-subprocess retries."""
    import os
    import pickle
    import subprocess
    import sys
    import tempfile
    import time

    in_maps, host_ctx = _prep_core_inputs(inputs)
    with tempfile.TemporaryDirectory() as td:
        in_pkl = os.path.join(td, "in.pkl")
        out_pkl = os.path.join(td, "out.pkl")
        with open(in_pkl, "wb") as f:
            pickle.dump({"in_maps": in_maps,
                         "with_bias": host_ctx["with_bias"]}, f)
        # fast path: run in-process (build is cached for repeat calls)
        try:
            results = _run_device(in_maps, host_ctx["with_bias"])
            return _assemble(results, host_ctx)
        except Exception as e:
            last_err = e
        # device fault: the owning process can't recover the accelerator;
        # retry in fresh subprocesses after letting the pool reset
        for attempt in range(2):
            time.sleep(75)
            try:
                proc = subprocess.run(
                    [sys.executable, os.path.abspath(__file__),
                     "--worker", in_pkl, out_pkl],
                    timeout=1800, capture_output=True, text=True)
                if proc.returncode == 0 and os.path.exists(out_pkl):
                    with open(out_pkl, "rb") as f:
                        results = pickle.load(f)
                    return _assemble(results, host_ctx)
                last_err = RuntimeError(
                    f"worker rc={proc.returncode}: {proc.stderr[-2000:]}")
            except Exception as e:
                last_err = e
        raise last_err


if __name__ == "__main__":
    import sys
    if len(sys.argv) == 4 and sys.argv[1] == "--worker":
        _worker_main(sys.argv[2], sys.argv[3])
